# revision 51
# baseline (speedup 1.0000x reference)
"""nn_EncoderModel: 2-layer LSTM encoder (B=128, T=512, E=256, H=1024)
on 8 trn2 NeuronCores — v3.

Hidden-dim model parallelism (core k owns h-dims [128k,128(k+1)) of
both layers), layer 1 lagged one step behind layer 0. The batch is
split into TWO independent 64-row streams: each stream runs its own
merged AllGather per step ([h0(t); h1(t-1)] in bf16, 32KB payload), so
one stream's AG latency hides under the other stream's compute and the
per-step serial chain no longer pays the collective round trip.

No per-step length masking: states evolve freely past each sequence's
end and the output is captured at its firing step
(outacc += msel_t * h1), mathematically identical to dynamic_rnn's
freeze-and-read-last.

Embedding lookup folds into the layer-0 matmul via the one-hot trick;
the one-hot is built once per step for the full batch (DVE is_equal +
PE transpose) and column-sliced per stream as the stationary operand.

Queues: PE = matmuls + transposes (all MMs of both streams emitted
before the transposes so PE never stalls waiting on a cell); ACT =
activations + ohT copy; DVE = cell elementwise + stage copies; gpsimd
= collectives (AG_s0 then AG_s1 per step); SP = cin DMAs + readbacks
(both cins emitted before both readbacks to avoid head-of-line
blocking behind an AG-completion wait).
"""

from contextlib import ExitStack

import numpy as np

import concourse.bass as bass
import concourse.mybir as mybir
import concourse.tile as tile
from concourse import bacc
from concourse.bass_utils import run_bass_kernel_spmd

F32 = mybir.dt.float32
F32R = mybir.dt.float32r
BF16 = mybir.dt.bfloat16
AF = mybir.ActivationFunctionType
ALU = mybir.AluOpType

B = 128      # batch (full, on every core)
BH = 64      # half batch per stream
E = 256      # embedding dim
H = 1024     # hidden
V = 128      # vocab
T = 512      # timesteps
HSL = 128    # hidden slice per core
G = 4 * HSL  # gate cols per core = 512
NCORES = 8

EXCH = "bf16"   # "f32r" | "bf16": dtype of weights + h-exchange payload

# gate order within each core's G columns: (i, o, f, j)
# reference order in W is (i, j, f, o) -> permutation of source blocks:
GATE_PERM = [0, 3, 2, 1]
CI, CO, CF, CJ = 0, 128, 256, 384  # column offsets of i/o/f/j blocks


def _np_dt(exch):
    if exch == "bf16":
        return mybir.dt.np(BF16)
    return np.float32


def _host_prep(inputs, exch=None):
    """Slice/transform full inputs into 8 per-core input maps."""
    exch = EXCH if exch is None else exch
    wdt = _np_dt(exch)
    ib = np.asarray(inputs["input_batch"])            # [B, T] int32
    lens = np.asarray(inputs["input_lengths"])        # [B]
    emb = np.asarray(inputs["char_embeddings"], dtype=np.float32)  # [V, E]
    W0 = np.asarray(inputs["W0"], dtype=np.float32)   # [E+H, 4H]
    b0 = np.asarray(inputs["b0"], dtype=np.float32)
    W1 = np.asarray(inputs["W1"], dtype=np.float32)   # [2H, 4H]
    b1 = np.asarray(inputs["b1"], dtype=np.float32)

    def gate_cols(W, k):
        return np.concatenate(
            [W[:, g * H + k * HSL: g * H + (k + 1) * HSL] for g in GATE_PERM],
            axis=1,
        )

    def gate_cols_b(b, k):
        return np.concatenate(
            [b[g * H + k * HSL: g * H + (k + 1) * HSL] for g in GATE_PERM]
        )

    tok_f32 = ib[:, :T].astype(np.float32)            # [B, T]
    iota_free = np.tile(np.arange(V, dtype=np.float32)[None, :], (B, 1))
    # capture mask: fires once, at each sequence's last valid step
    msel = (np.arange(T)[None, :] == (lens[:, None] - 1)).astype(np.float32)
    ident = np.eye(128, dtype=np.float32)

    in_maps = []
    for k in range(NCORES):
        W0c = gate_cols(W0, k)                        # [E+H, 512] (i,o,f,j)
        b0c = gate_cols_b(b0, k).copy()               # [512]
        b0c[CF:CF + HSL] += 1.0                       # forget bias layer 0
        ewb = emb @ W0c[:E] + b0c[None, :]            # [V, 512]
        w0h = W0c[E:]                                 # [1024, 512]
        w0h_t = np.concatenate(
            [w0h[j * 128: (j + 1) * 128] for j in range(8)], axis=1)
        W1c = gate_cols(W1, k)                        # [2048, 512]
        b1c = gate_cols_b(b1, k)
        # k-tile blocks: 0..7 = nh0 rows, 8..15 = h1 rows
        w1_t = np.concatenate(
            [W1c[j * 128: (j + 1) * 128] for j in range(16)], axis=1)
        b1_full = np.tile(b1c[None, :], (128, 1)).astype(np.float32)
        in_maps.append({
            "ewb": ewb.astype(wdt),
            "w0h": w0h_t.astype(wdt),
            "w1": w1_t.astype(wdt),
            "b1full": b1_full,
            "tok": tok_f32,
            "iotaf": iota_free,
            "msela": np.ascontiguousarray(msel[0:BH]),
            "mselb": np.ascontiguousarray(msel[BH:2 * BH]),
            "ident": ident,
        })
    has_b1 = bool(np.any(b1 != 0.0))
    return in_maps, has_b1


def build_kernel(has_b1=False, ag_mode="dual", exch=None):
    """Build + compile the SPMD Bass kernel for all 8 cores.

    ag_mode: "dual" (real AllGathers) / "fake" / "local" (no collectives;
    local copies — WRONG results, compute-floor measurement only).
    """
    exch = EXCH if exch is None else exch
    XDT = BF16 if exch == "bf16" else F32R
    nc = bacc.Bacc("TRN2", target_bir_lowering=False, debug=False,
                   num_devices=NCORES)

    # ---- I/O ----
    d_ewb = nc.dram_tensor("ewb", [V, G], XDT, kind="ExternalInput")
    d_w0h = nc.dram_tensor("w0h", [128, 8 * G], XDT, kind="ExternalInput")
    d_w1 = nc.dram_tensor("w1", [128, 16 * G], XDT, kind="ExternalInput")
    d_b1 = nc.dram_tensor("b1full", [128, G], F32, kind="ExternalInput")
    d_tok = nc.dram_tensor("tok", [B, T], F32, kind="ExternalInput")
    d_iota = nc.dram_tensor("iotaf", [B, V], F32, kind="ExternalInput")
    d_msela = nc.dram_tensor("msela", [BH, T], F32, kind="ExternalInput")
    d_mselb = nc.dram_tensor("mselb", [BH, T], F32, kind="ExternalInput")
    d_ident = nc.dram_tensor("ident", [128, 128], F32, kind="ExternalInput")
    d_out = nc.dram_tensor("out", [B, HSL], F32, kind="ExternalOutput")

    # ---- persistent SBUF ----
    sb_ewb = nc.alloc_sbuf_tensor("sb_ewb", [V, G], XDT)
    sb_w0h = nc.alloc_sbuf_tensor("sb_w0h", [128, 8 * G], XDT)
    sb_w1 = nc.alloc_sbuf_tensor("sb_w1", [128, 16 * G], XDT)
    sb_b1 = nc.alloc_sbuf_tensor("sb_b1", [128, G], F32)
    sb_tok = nc.alloc_sbuf_tensor("sb_tok", [B, T], F32)
    sb_iota = nc.alloc_sbuf_tensor("sb_iota", [B, V], F32)
    sb_msel = [nc.alloc_sbuf_tensor("sb_msela", [BH, T], F32),
               nc.alloc_sbuf_tensor("sb_mselb", [BH, T], F32)]
    sb_ident = nc.alloc_sbuf_tensor("sb_ident", [128, 128], F32)
    zrow = nc.alloc_sbuf_tensor("zrow", [HSL, B], F32)
    # per-stream states, all based at partition 0
    c0s = [nc.alloc_sbuf_tensor(f"c0_{s}", [BH, HSL], F32) for s in (0, 1)]
    c1s = [nc.alloc_sbuf_tensor(f"c1_{s}", [BH, HSL], F32) for s in (0, 1)]
    h0s = [nc.alloc_sbuf_tensor(f"h0_{s}", [BH, HSL], F32) for s in (0, 1)]
    h1s = [nc.alloc_sbuf_tensor(f"h1_{s}", [BH, HSL], F32) for s in (0, 1)]
    oas = [nc.alloc_sbuf_tensor(f"oa_{s}", [BH, HSL], F32) for s in (0, 1)]

    with tile.TileContext(nc) as tc, ExitStack() as ctx:
        # ---- load weights/constants ----
        for sb, d in [(sb_ewb, d_ewb), (sb_w0h, d_w0h), (sb_w1, d_w1),
                      (sb_b1, d_b1), (sb_tok, d_tok), (sb_iota, d_iota),
                      (sb_msel[0], d_msela), (sb_msel[1], d_mselb),
                      (sb_ident, d_ident)]:
            nc.sync.dma_start(sb[:], d[:])
        for st in c0s + c1s + h0s + h1s + oas + [zrow]:
            nc.vector.memset(st[:], 0.0)

        # ---- pools ----
        ps_z0 = ctx.enter_context(tc.tile_pool(name="psz0", bufs=1, space="PSUM"))
        ps_z1 = ctx.enter_context(tc.tile_pool(name="psz1", bufs=1, space="PSUM"))
        ps_tp = ctx.enter_context(tc.tile_pool(name="pstp", bufs=2, space="PSUM"))
        pool = ctx.enter_context(tc.tile_pool(name="work", bufs=3))
        rp0 = ctx.enter_context(tc.tile_pool(name="recv0", bufs=2))
        dram = ctx.enter_context(tc.tile_pool(name="dram", bufs=2, space="DRAM"))

        def do_ag(cin, cout):
            if ag_mode == "local":
                nc.gpsimd.dma_start(cout[0:2 * HSL, :], cin[:])
            elif ag_mode == "fake":
                for j in range(NCORES):
                    nc.gpsimd.dma_start(
                        cout[j * 2 * HSL:(j + 1) * 2 * HSL, :], cin[:])
            else:
                nc.gpsimd.collective_compute(
                    "AllGather", ALU.bypass,
                    replica_groups=[list(range(NCORES))],
                    ins=[cin[:].opt()], outs=[cout[:].opt()],
                )

        def cell(z, cst, hst, layer, s):
            """Pure LSTM cell on PSUM gates z [BH, G] in (i,o,f,j) order;
            updates cst/hst in place (no length masking)."""
            sig = pool.tile([BH, 3 * HSL], F32, tag=f"sig{layer}{s}")
            if layer == 0:
                # f-bias folded into EWb: one fused sigmoid over i|o|f
                nc.scalar.activation(sig[:], z[:, 0:CJ], AF.Sigmoid)
            else:
                nc.scalar.activation(sig[:, 0:CF], z[:, 0:CF], AF.Sigmoid)
                nc.scalar.activation(sig[:, CF:CJ], z[:, CF:CJ], AF.Sigmoid,
                                     bias=1.0)
            tanj = pool.tile([BH, HSL], F32, tag=f"tanj{layer}{s}")
            nc.scalar.activation(tanj[:], z[:, CJ:CJ + HSL], AF.Tanh)
            # c = c*sigf + sigi*tanj
            u = pool.tile([BH, HSL], F32, tag=f"u{layer}{s}")
            nc.vector.tensor_mul(u[:], sig[:, 0:CO], tanj[:])
            cm = pool.tile([BH, HSL], F32, tag=f"cm{layer}{s}")
            nc.vector.tensor_mul(cm[:], cst, sig[:, CF:CJ])
            nc.vector.tensor_add(cst, cm[:], u[:])
            # h = tanh(c) * sigo
            tanc = pool.tile([BH, HSL], F32, tag=f"tanc{layer}{s}")
            nc.scalar.activation(tanc[:], cst, AF.Tanh)
            nc.vector.tensor_mul(hst, tanc[:], sig[:, CO:CF])

        # per-stream merged exchange: AG_s carries [h0_s(t); h1_s(t-1)].
        # rh_s block layout: col (2j)*BH: core j's h0; (2j+1)*BH: h1.
        rh_l = [None, None]

        def rblk0(s, j):
            return rh_l[s][:, (2 * j) * BH:(2 * j) * BH + BH]

        def rblk1(s, j):
            return rh_l[s][:, (2 * j + 1) * BH:(2 * j + 1) * BH + BH]

        for t in range(T + 1):
            # ---- one-hot for x_t (full batch, shared by both streams) --
            if t < T:
                ohbt = pool.tile([B, V], F32, tag="ohbt")
                nc.vector.tensor_scalar(
                    ohbt[:], sb_iota[:], sb_tok[:, t:t + 1], None,
                    ALU.is_equal)
                poh = ps_tp.tile([V, B], F32, tag="poh")
                nc.tensor.transpose(poh[:], ohbt[:], sb_ident[:])
                ohT = pool.tile([V, B], XDT, tag="ohT")
                nc.scalar.copy(ohT[:], poh[:])

            # ---- phase 1: all matmuls of both streams ----
            z0l, z1l = [None, None], [None, None]
            for s in (0, 1):
                if t < T:
                    z0 = ps_z0.tile([BH, G], F32, tag=f"z0{s}")
                    z0l[s] = z0
                    nc.tensor.matmul(
                        z0[:], ohT[:, s * BH:(s + 1) * BH], sb_ewb[:],
                        start=True, stop=(t == 0))
                    if t > 0:
                        for j in range(8):
                            nc.tensor.matmul(
                                z0[:], rblk0(s, j),
                                sb_w0h[:, j * G:(j + 1) * G],
                                start=False, stop=(j == 7))
                if t >= 1:
                    z1 = ps_z1.tile([BH, G], F32, tag=f"z1{s}")
                    z1l[s] = z1
                    for j in range(8):
                        nc.tensor.matmul(
                            z1[:], rblk0(s, j),
                            sb_w1[:, j * G:(j + 1) * G],
                            start=(j == 0), stop=(t == 1 and j == 7))
                    if t >= 2:
                        for j in range(8):
                            nc.tensor.matmul(
                                z1[:], rblk1(s, j),
                                sb_w1[:, (8 + j) * G:(9 + j) * G],
                                start=False, stop=(j == 7),
                                skip_group_check=True)

            # ---- phase 2: cells + capture ----
            for s in (0, 1):
                if t < T:
                    cell(z0l[s], c0s[s][:], h0s[s][:], 0, s)
                if t >= 1:
                    if has_b1:
                        zb = pool.tile([BH, G], F32, tag=f"zb{s}")
                        nc.vector.tensor_add(zb[:], z1l[s][:],
                                             sb_b1[0:BH, :])
                        z1ap = zb
                    else:
                        z1ap = z1l[s]
                    cell(z1ap, c1s[s][:], h1s[s][:], 1, s)
                    # capture h1(t-1) into the output at its firing step
                    nc.vector.scalar_tensor_tensor(
                        oas[s][:], h1s[s][:], sb_msel[s][:, t - 1:t],
                        oas[s][:], ALU.mult, ALU.add)

            # ---- phase 3: transpose + stage + cin + AG per stream ----
            stgs, couts = [None, None], [None, None]
            for s in (0, 1):
                if t < T:
                    stg = pool.tile([HSL, 2 * BH], XDT, tag=f"stg{s}")
                    stgs[s] = stg
                    tp0 = ps_tp.tile([HSL, BH], F32, tag="tp")
                    nc.tensor.transpose(tp0[:], h0s[s][:],
                                        sb_ident[0:BH, 0:BH])
                    nc.vector.tensor_copy(stg[:, 0:BH], tp0[:])
                    if t >= 1:
                        tp1 = ps_tp.tile([HSL, BH], F32, tag="tp")
                        nc.tensor.transpose(tp1[:], h1s[s][:],
                                            sb_ident[0:BH, 0:BH])
                        nc.vector.tensor_copy(stg[:, BH:2 * BH], tp1[:])
                    else:
                        nc.vector.tensor_copy(stg[:, BH:2 * BH],
                                              zrow[:, 0:BH])
                    cin = dram.tile([2 * HSL, BH], XDT, tag=f"cin{s}")
                    nc.sync.dma_start(
                        cin[:].rearrange("(q p) b -> p q b", q=2),
                        stg[:].rearrange("p (q b) -> p q b", q=2))
                    cout = dram.tile([NCORES * 2 * HSL, BH], XDT,
                                     tag=f"cout{s}")
                    couts[s] = cout
                    do_ag(cin, cout)

            # ---- phase 4: readbacks (after both cins are enqueued) ----
            for s in (0, 1):
                if t < T:
                    rh = rp0.tile([128, NCORES * 2 * BH], XDT, tag=f"rh{s}")
                    rh_l[s] = rh
                    nc.sync.dma_start(
                        rh[:].rearrange("p (j q b) -> p j q b",
                                        j=NCORES, q=2),
                        couts[s][:].rearrange("(j q p) b -> p j q b",
                                              j=NCORES, q=2))

        # ---- output ----
        nc.sync.dma_start(d_out[0:BH, :], oas[0][:])
        nc.sync.dma_start(d_out[BH:2 * BH, :], oas[1][:])

    nc.compile()
    return nc


_CACHE = {}


def kernel(**inputs) -> np.ndarray:
    """Full-input entry point: returns [B, H] fp32 encoder output."""
    in_maps, has_b1 = _host_prep(inputs)
    key = ("nc", has_b1, EXCH)
    if key not in _CACHE:
        _CACHE[key] = build_kernel(has_b1=has_b1)
    nc = _CACHE[key]
    res = run_bass_kernel_spmd(nc, in_maps, core_ids=list(range(NCORES)))
    out = np.concatenate(
        [res.results[k]["out"] for k in range(NCORES)], axis=1)
    return out.astype(np.float32)


# revision 54
# speedup vs baseline: 1.0301x; 1.0301x over previous
"""nn_EncoderModel: 2-layer LSTM encoder (B=128, T=512, E=256, H=1024)
on 8 trn2 NeuronCores — v2.3.

Hidden-dim model parallelism (core k owns h-dims [128k,128(k+1)) of
both layers), layer 1 lagged one step behind layer 0, ONE merged
AllGather per step carrying [h0(t); h1(t-1)] in bf16 (the AG here is
latency+size bound: 32KB ~6us vs 128KB ~14us serial). No per-step
length masking: states evolve freely past each sequence's end and the
output is captured at its firing step (outacc += msel_t * h1), which
is mathematically identical to dynamic_rnn's freeze-and-read-last.

Embedding lookup folds into the layer-0 matmul via the one-hot trick;
the one-hot is built in [B, V] layout (DVE is_equal), PE-transposed to
[V, B], and used as the stationary operand of the EWb matmul (b0 and
the layer-0 forget bias are folded into EWb on the host).

Queues: PE = matmuls + state transposes; ACT = activations + ohT copy;
DVE = cell elementwise + stage copies; gpsimd = collectives; SP = cin
DMA + readbacks.
"""

from contextlib import ExitStack

import numpy as np

import concourse.bass as bass
import concourse.mybir as mybir
import concourse.tile as tile
from concourse import bacc
from concourse.bass_utils import run_bass_kernel_spmd

F32 = mybir.dt.float32
F32R = mybir.dt.float32r
BF16 = mybir.dt.bfloat16
AF = mybir.ActivationFunctionType
ALU = mybir.AluOpType

B = 128      # batch (full, on every core)
E = 256      # embedding dim
H = 1024     # hidden
V = 128      # vocab
T = 512      # timesteps
HSL = 128    # hidden slice per core
G = 4 * HSL  # gate cols per core = 512
NCORES = 8

EXCH = "bf16"   # "f32r" | "bf16": dtype of weights + h-exchange payload

# gate order within each core's G columns: (i, o, f, j)
# reference order in W is (i, j, f, o) -> permutation of source blocks:
GATE_PERM = [0, 3, 2, 1]
CI, CO, CF, CJ = 0, 128, 256, 384  # column offsets of i/o/f/j blocks


def _np_dt(exch):
    if exch == "bf16":
        return mybir.dt.np(BF16)
    return np.float32


def _host_prep(inputs, exch=None):
    """Slice/transform full inputs into 8 per-core input maps."""
    exch = EXCH if exch is None else exch
    wdt = _np_dt(exch)
    ib = np.asarray(inputs["input_batch"])            # [B, T] int32
    lens = np.asarray(inputs["input_lengths"])        # [B]
    emb = np.asarray(inputs["char_embeddings"], dtype=np.float32)  # [V, E]
    W0 = np.asarray(inputs["W0"], dtype=np.float32)   # [E+H, 4H]
    b0 = np.asarray(inputs["b0"], dtype=np.float32)
    W1 = np.asarray(inputs["W1"], dtype=np.float32)   # [2H, 4H]
    b1 = np.asarray(inputs["b1"], dtype=np.float32)

    def gate_cols(W, k):
        return np.concatenate(
            [W[:, g * H + k * HSL: g * H + (k + 1) * HSL] for g in GATE_PERM],
            axis=1,
        )

    def gate_cols_b(b, k):
        return np.concatenate(
            [b[g * H + k * HSL: g * H + (k + 1) * HSL] for g in GATE_PERM]
        )

    tok_f32 = ib[:, :T].astype(np.float32)            # [B, T]
    iota_free = np.tile(np.arange(V, dtype=np.float32)[None, :], (B, 1))
    # capture mask: fires once, at each sequence's last valid step
    msel = (np.arange(T)[None, :] == (lens[:, None] - 1)).astype(np.float32)
    iota_p = np.arange(V, dtype=np.float32)[:, None]  # [V, 1]
    ident = np.eye(128, dtype=np.float32)

    in_maps = []
    for k in range(NCORES):
        W0c = gate_cols(W0, k)                        # [E+H, 512] (i,o,f,j)
        b0c = gate_cols_b(b0, k).copy()               # [512]
        b0c[CF:CF + HSL] += 1.0                       # forget bias layer 0
        ewb = emb @ W0c[:E] + b0c[None, :]            # [V, 512]
        w0h = W0c[E:]                                 # [1024, 512]
        w0h_t = np.concatenate(
            [w0h[j * 128: (j + 1) * 128] for j in range(8)], axis=1)
        W1c = gate_cols(W1, k)                        # [2048, 512]
        b1c = gate_cols_b(b1, k)
        # k-tile blocks: 0..7 = nh0 rows, 8..15 = h1 rows
        w1_t = np.concatenate(
            [W1c[j * 128: (j + 1) * 128] for j in range(16)], axis=1)
        b1_full = np.tile(b1c[None, :], (128, 1)).astype(np.float32)
        in_maps.append({
            "ewb": ewb.astype(wdt),
            "w0h": w0h_t.astype(wdt),
            "w1": w1_t.astype(wdt),
            "b1full": b1_full,
            "tok": tok_f32,
            "iotaf": iota_free,
            "msel": msel,
            "iotap": iota_p,
            "ident": ident,
        })
    has_b1 = bool(np.any(b1 != 0.0))
    return in_maps, has_b1


def build_kernel(has_b1=False, ag_mode="dual", exch=None):
    """Build + compile the SPMD Bass kernel for all 8 cores.

    ag_mode: "dual" (two staggered AGs/step) or "fake" (no collectives;
    local copies — WRONG results, compute-floor measurement only).
    """
    exch = EXCH if exch is None else exch
    XDT = BF16 if exch == "bf16" else F32R
    nc = bacc.Bacc("TRN2", target_bir_lowering=False, debug=False,
                   num_devices=NCORES)

    # ---- I/O ----
    d_ewb = nc.dram_tensor("ewb", [V, G], XDT, kind="ExternalInput")
    d_w0h = nc.dram_tensor("w0h", [128, 8 * G], XDT, kind="ExternalInput")
    d_w1 = nc.dram_tensor("w1", [128, 16 * G], XDT, kind="ExternalInput")
    d_b1 = nc.dram_tensor("b1full", [128, G], F32, kind="ExternalInput")
    d_tok = nc.dram_tensor("tok", [B, T], F32, kind="ExternalInput")
    d_iota = nc.dram_tensor("iotaf", [B, V], F32, kind="ExternalInput")
    d_msel = nc.dram_tensor("msel", [B, T], F32, kind="ExternalInput")
    d_iotap = nc.dram_tensor("iotap", [V, 1], F32, kind="ExternalInput")
    d_ident = nc.dram_tensor("ident", [128, 128], F32, kind="ExternalInput")
    d_out = nc.dram_tensor("out", [B, HSL], F32, kind="ExternalOutput")

    # ---- persistent SBUF ----
    sb_ewb = nc.alloc_sbuf_tensor("sb_ewb", [V, G], XDT)
    sb_w0h = nc.alloc_sbuf_tensor("sb_w0h", [128, 8 * G], XDT)
    sb_w1 = nc.alloc_sbuf_tensor("sb_w1", [128, 16 * G], XDT)
    sb_b1 = nc.alloc_sbuf_tensor("sb_b1", [128, G], F32)
    sb_tok = nc.alloc_sbuf_tensor("sb_tok", [B, T], F32)
    sb_iota = nc.alloc_sbuf_tensor("sb_iota", [B, V], F32)
    sb_msel = nc.alloc_sbuf_tensor("sb_msel", [B, T], F32)
    sb_iotap = nc.alloc_sbuf_tensor("sb_iotap", [V, 1], F32)
    sb_ident = nc.alloc_sbuf_tensor("sb_ident", [128, 128], F32)
    c0 = nc.alloc_sbuf_tensor("c0", [B, HSL], F32)
    c1 = nc.alloc_sbuf_tensor("c1", [B, HSL], F32)
    h0bt = nc.alloc_sbuf_tensor("h0bt", [B, HSL], F32)
    h1bt = nc.alloc_sbuf_tensor("h1bt", [B, HSL], F32)
    zrow = nc.alloc_sbuf_tensor("zrow", [HSL, B], F32)
    outacc = nc.alloc_sbuf_tensor("outacc", [B, HSL], F32)

    with tile.TileContext(nc) as tc, ExitStack() as ctx:
        # ---- load weights/constants ----
        for sb, d in [(sb_ewb, d_ewb), (sb_w0h, d_w0h), (sb_w1, d_w1),
                      (sb_b1, d_b1), (sb_tok, d_tok), (sb_iota, d_iota),
                      (sb_msel, d_msel), (sb_iotap, d_iotap),
                      (sb_ident, d_ident)]:
            nc.sync.dma_start(sb[:], d[:])
        for st in (c0, c1, h0bt, h1bt, zrow, outacc):
            nc.vector.memset(st[:], 0.0)

        # ---- pools ----
        ps_z0 = ctx.enter_context(tc.tile_pool(name="psz0", bufs=2, space="PSUM"))
        ps_z1 = ctx.enter_context(tc.tile_pool(name="psz1", bufs=2, space="PSUM"))
        ps_tp = ctx.enter_context(tc.tile_pool(name="pstp", bufs=1, space="PSUM"))
        pool = ctx.enter_context(tc.tile_pool(name="work", bufs=3))
        rp0 = ctx.enter_context(tc.tile_pool(name="recv0", bufs=2))
        dram = ctx.enter_context(tc.tile_pool(name="dram", bufs=2, space="DRAM"))

        def do_ag(cin, cout):
            if ag_mode == "local":
                # timing-only: no exchange; block 0 fed from local cin
                nc.gpsimd.dma_start(cout[0:2 * HSL, :], cin[:])
            elif ag_mode == "fake":
                for j in range(NCORES):
                    nc.gpsimd.dma_start(
                        cout[j * 2 * HSL:(j + 1) * 2 * HSL, :], cin[:])
            else:
                nc.gpsimd.collective_compute(
                    "AllGather", ALU.bypass,
                    replica_groups=[list(range(NCORES))],
                    ins=[cin[:].opt()], outs=[cout[:].opt()],
                )

        # The two cells of one iteration (layer 0 of step t, layer 1 of
        # step t-1) are independent chains; their phases are emitted
        # interleaved so the in-order ACT/DVE queues never stall behind a
        # cross-engine wait belonging to the other cell (head-of-line
        # blocking). Phases: A = gate activations (ACT), B = c-update
        # (DVE), C = tanh(c) (ACT), D = h (DVE), then transposes (PE),
        # stage copies + capture (DVE, last — off the exchange path).

        # merged exchange: one AG/step carries [h0(t); h1(t-1)].
        # rh block layout: col (2j)*B..: core j's h0 slice; (2j+1)*B..: h1.
        def rblk0(j):
            return rh[:, (2 * j) * B:(2 * j) * B + B]

        def rblk1(j):
            return rh[:, (2 * j + 1) * B:(2 * j + 1) * B + B]

        rh = None

        for t in range(T + 1):
            # ---- one-hot for x_t, direct in [V, B] layout ----
            if t < T:
                ohbt = pool.tile([B, V], F32, tag="ohbt")
                nc.vector.tensor_scalar(
                    ohbt[:], sb_iota[:], sb_tok[:, t:t + 1], None,
                    ALU.is_equal)
                poh = ps_tp.tile([V, B], F32, tag="poh")
                nc.tensor.transpose(poh[:], ohbt[:], sb_ident[:])
                ohT = pool.tile([V, B], XDT, tag="ohT")
                nc.scalar.copy(ohT[:], poh[:])

                # ---- z0(t) = oh @ EWb + h0(t-1) @ W0h ----
                z0 = ps_z0.tile([B, G], F32, tag="z0")
                nc.tensor.matmul(z0[:], ohT[:], sb_ewb[:],
                                 start=True, stop=(t == 0))
                if t > 0:
                    for j in range(8):
                        nc.tensor.matmul(
                            z0[:], rblk0(j),
                            sb_w0h[:, j * G:(j + 1) * G],
                            start=False, stop=(j == 7))

            # ---- z1(t-1) = nh0(t-1) @ W1a + h1(t-2) @ W1b ----
            if t >= 1:
                z1 = ps_z1.tile([B, G], F32, tag="z1")
                for j in range(8):
                    nc.tensor.matmul(
                        z1[:], rblk0(j),
                        sb_w1[:, j * G:(j + 1) * G],
                        start=(j == 0), stop=(t == 1 and j == 7))
                if t >= 2:
                    for j in range(8):
                        nc.tensor.matmul(
                            z1[:], rblk1(j),
                            sb_w1[:, (8 + j) * G:(9 + j) * G],
                            start=False, stop=(j == 7),
                            skip_group_check=True)

            # ---- cells, phase-interleaved ----
            do0, do1 = t < T, t >= 1
            if do1:
                if has_b1:
                    zb = pool.tile([B, G], F32, tag="zb")
                    nc.vector.tensor_add(zb[:], z1[:], sb_b1[:])
                    z1ap = zb
                else:
                    z1ap = z1
            # phase A: gate activations (f-bias of layer 0 folded in EWb)
            if do0:
                sig0 = pool.tile([B, 3 * HSL], F32, tag="sig0")
                nc.scalar.activation(sig0[:], z0[:, 0:CJ], AF.Sigmoid)
                tanj0 = pool.tile([B, HSL], F32, tag="tanj0")
                nc.scalar.activation(tanj0[:], z0[:, CJ:CJ + HSL], AF.Tanh)
            if do1:
                sig1 = pool.tile([B, 3 * HSL], F32, tag="sig1")
                nc.scalar.activation(sig1[:, 0:CF], z1ap[:, 0:CF],
                                     AF.Sigmoid)
                nc.scalar.activation(sig1[:, CF:CJ], z1ap[:, CF:CJ],
                                     AF.Sigmoid, bias=1.0)
                tanj1 = pool.tile([B, HSL], F32, tag="tanj1")
                nc.scalar.activation(tanj1[:], z1ap[:, CJ:CJ + HSL],
                                     AF.Tanh)
            # phase B: c = c*sigf + sigi*tanj
            if do0:
                u0 = pool.tile([B, HSL], F32, tag="u0")
                nc.vector.tensor_mul(u0[:], sig0[:, 0:CO], tanj0[:])
                cm0 = pool.tile([B, HSL], F32, tag="cm0")
                nc.vector.tensor_mul(cm0[:], c0[:], sig0[:, CF:CJ])
                nc.vector.tensor_add(c0[:], cm0[:], u0[:])
            if do1:
                u1 = pool.tile([B, HSL], F32, tag="u1")
                nc.vector.tensor_mul(u1[:], sig1[:, 0:CO], tanj1[:])
                cm1 = pool.tile([B, HSL], F32, tag="cm1")
                nc.vector.tensor_mul(cm1[:], c1[:], sig1[:, CF:CJ])
                nc.vector.tensor_add(c1[:], cm1[:], u1[:])
            # phase C: tanh(c)
            if do0:
                tanc0 = pool.tile([B, HSL], F32, tag="tanc0")
                nc.scalar.activation(tanc0[:], c0[:], AF.Tanh)
            if do1:
                tanc1 = pool.tile([B, HSL], F32, tag="tanc1")
                nc.scalar.activation(tanc1[:], c1[:], AF.Tanh)
            # phase D: h = tanh(c)*sigo, then transposes + staging
            if do0:
                nc.vector.tensor_mul(h0bt[:], tanc0[:], sig0[:, CO:CF])
            if do1:
                nc.vector.tensor_mul(h1bt[:], tanc1[:], sig1[:, CO:CF])
            if do0:
                stg = pool.tile([HSL, 2 * B], XDT, tag="stg")
                tp0 = ps_tp.tile([HSL, B], F32, tag="tp0")
                nc.tensor.transpose(tp0[:], h0bt[:], sb_ident[:])
                if do1:
                    tp1 = ps_tp.tile([HSL, B], F32, tag="tp1")
                    nc.tensor.transpose(tp1[:], h1bt[:], sb_ident[:])
                nc.vector.tensor_copy(stg[:, 0:B], tp0[:])
                if do1:
                    nc.vector.tensor_copy(stg[:, B:2 * B], tp1[:])
                else:
                    nc.vector.tensor_copy(stg[:, B:2 * B], zrow[:])
            # capture h1(t-1) into the output at its firing step
            # (emitted last on DVE — not on the exchange critical path)
            if do1:
                nc.vector.scalar_tensor_tensor(
                    outacc[:], h1bt[:], sb_msel[:, t - 1:t], outacc[:],
                    ALU.mult, ALU.add)

            # ---- single AG + readback ----
            if t < T:
                cin = dram.tile([2 * HSL, B], XDT, tag="cin")
                nc.sync.dma_start(
                    cin[:].rearrange("(s p) b -> p s b", s=2),
                    stg[:].rearrange("p (s b) -> p s b", s=2))
                cout = dram.tile([NCORES * 2 * HSL, B], XDT, tag="cout")
                do_ag(cin, cout)
                rh = rp0.tile([128, NCORES * 2 * HSL], XDT, tag="rh")
                nc.sync.dma_start(
                    rh[:].rearrange("p (j s b) -> p j s b", j=NCORES, s=2),
                    cout[:].rearrange("(j s p) b -> p j s b", j=NCORES, s=2))

        # ---- output ----
        nc.sync.dma_start(d_out[:], outacc[:])

    nc.compile()
    return nc


_CACHE = {}


def kernel(**inputs) -> np.ndarray:
    """Full-input entry point: returns [B, H] fp32 encoder output."""
    in_maps, has_b1 = _host_prep(inputs)
    key = ("nc", has_b1, EXCH)
    if key not in _CACHE:
        _CACHE[key] = build_kernel(has_b1=has_b1)
    nc = _CACHE[key]
    res = run_bass_kernel_spmd(nc, in_maps, core_ids=list(range(NCORES)))
    out = np.concatenate(
        [res.results[k]["out"] for k in range(NCORES)], axis=1)
    return out.astype(np.float32)



# revision 56
# speedup vs baseline: 1.0442x; 1.0136x over previous
"""nn_EncoderModel: 2-layer LSTM encoder (B=128, T=512, E=256, H=1024)
on 8 trn2 NeuronCores — v2.6.

Hidden-dim model parallelism (core k owns h-dims [128k,128(k+1)) of
both layers), layer 1 lagged one step behind layer 0, ONE merged
AllGather per step carrying [h0(t); h1(t-1)] in bf16 (the AG here is
latency+size bound: 32KB ~6us vs 128KB ~14us serial). No per-step
length masking: states evolve freely past each sequence's end and the
output is captured at its firing step (outacc += msel_t * h1), which
is mathematically identical to dynamic_rnn's freeze-and-read-last.

Embedding lookup folds into the layer-0 matmul via the one-hot trick;
the one-hot is built in [B, V] layout (DVE is_equal), PE-transposed to
[V, B], and used as the stationary operand of the EWb matmul (b0 and
the layer-0 forget bias are folded into EWb on the host).

Queues: PE = matmuls + state transposes; ACT = activations + ohT copy;
DVE = cell elementwise + stage copies; gpsimd = collectives; SP = cin
DMA + readbacks. The two cells of an iteration are emitted
phase-interleaved (see below) to avoid in-order-queue head-of-line
stalls.
"""

from contextlib import ExitStack

import numpy as np

import concourse.bass as bass
import concourse.mybir as mybir
import concourse.tile as tile
from concourse import bacc
from concourse.bass_utils import run_bass_kernel_spmd

F32 = mybir.dt.float32
F32R = mybir.dt.float32r
BF16 = mybir.dt.bfloat16
AF = mybir.ActivationFunctionType
ALU = mybir.AluOpType

B = 128      # batch (full, on every core)
E = 256      # embedding dim
H = 1024     # hidden
V = 128      # vocab
T = 512      # timesteps
HSL = 128    # hidden slice per core
G = 4 * HSL  # gate cols per core = 512
NCORES = 8

EXCH = "bf16"   # "f32r" | "bf16": dtype of weights + h-exchange payload

# gate order within each core's G columns: (i, o, f, j)
# reference order in W is (i, j, f, o) -> permutation of source blocks:
GATE_PERM = [0, 3, 2, 1]
CI, CO, CF, CJ = 0, 128, 256, 384  # column offsets of i/o/f/j blocks


def _np_dt(exch):
    if exch == "bf16":
        return mybir.dt.np(BF16)
    return np.float32


def _host_prep(inputs, exch=None):
    """Slice/transform full inputs into 8 per-core input maps."""
    exch = EXCH if exch is None else exch
    wdt = _np_dt(exch)
    ib = np.asarray(inputs["input_batch"])            # [B, T] int32
    lens = np.asarray(inputs["input_lengths"])        # [B]
    emb = np.asarray(inputs["char_embeddings"], dtype=np.float32)  # [V, E]
    W0 = np.asarray(inputs["W0"], dtype=np.float32)   # [E+H, 4H]
    b0 = np.asarray(inputs["b0"], dtype=np.float32)
    W1 = np.asarray(inputs["W1"], dtype=np.float32)   # [2H, 4H]
    b1 = np.asarray(inputs["b1"], dtype=np.float32)

    def gate_cols(W, k):
        return np.concatenate(
            [W[:, g * H + k * HSL: g * H + (k + 1) * HSL] for g in GATE_PERM],
            axis=1,
        )

    def gate_cols_b(b, k):
        return np.concatenate(
            [b[g * H + k * HSL: g * H + (k + 1) * HSL] for g in GATE_PERM]
        )

    tok_f32 = ib[:, :T].astype(np.float32)            # [B, T]
    iota_free = np.tile(np.arange(V, dtype=np.float32)[None, :], (B, 1))
    # capture mask: fires once, at each sequence's last valid step
    msel = (np.arange(T)[None, :] == (lens[:, None] - 1)).astype(np.float32)
    iota_p = np.arange(V, dtype=np.float32)[:, None]  # [V, 1]
    ident = np.eye(128, dtype=np.float32)

    in_maps = []
    for k in range(NCORES):
        W0c = gate_cols(W0, k)                        # [E+H, 512] (i,o,f,j)
        b0c = gate_cols_b(b0, k).copy()               # [512]
        b0c[CF:CF + HSL] += 1.0                       # forget bias layer 0
        ewb = emb @ W0c[:E] + b0c[None, :]            # [V, 512]
        w0h = W0c[E:]                                 # [1024, 512]
        w0h_t = np.concatenate(
            [w0h[j * 128: (j + 1) * 128] for j in range(8)], axis=1)
        W1c = gate_cols(W1, k)                        # [2048, 512]
        b1c = gate_cols_b(b1, k)
        # k-tile blocks: 0..7 = nh0 rows, 8..15 = h1 rows
        w1_t = np.concatenate(
            [W1c[j * 128: (j + 1) * 128] for j in range(16)], axis=1)
        b1_full = np.tile(b1c[None, :], (128, 1)).astype(np.float32)
        in_maps.append({
            "ewb": ewb.astype(wdt),
            "w0h": w0h_t.astype(wdt),
            "w1": w1_t.astype(wdt),
            "b1full": b1_full,
            "tok": tok_f32,
            "iotaf": iota_free,
            "msel": msel,
            "iotap": iota_p,
            "ident": ident,
        })
    has_b1 = bool(np.any(b1 != 0.0))
    return in_maps, has_b1


def build_kernel(has_b1=False, ag_mode="dual", exch=None):
    """Build + compile the SPMD Bass kernel for all 8 cores.

    ag_mode: "dual" (two staggered AGs/step) or "fake" (no collectives;
    local copies — WRONG results, compute-floor measurement only).
    """
    exch = EXCH if exch is None else exch
    XDT = BF16 if exch == "bf16" else F32R
    nc = bacc.Bacc("TRN2", target_bir_lowering=False, debug=False,
                   num_devices=NCORES)

    # ---- I/O ----
    d_ewb = nc.dram_tensor("ewb", [V, G], XDT, kind="ExternalInput")
    d_w0h = nc.dram_tensor("w0h", [128, 8 * G], XDT, kind="ExternalInput")
    d_w1 = nc.dram_tensor("w1", [128, 16 * G], XDT, kind="ExternalInput")
    d_b1 = nc.dram_tensor("b1full", [128, G], F32, kind="ExternalInput")
    d_tok = nc.dram_tensor("tok", [B, T], F32, kind="ExternalInput")
    d_iota = nc.dram_tensor("iotaf", [B, V], F32, kind="ExternalInput")
    d_msel = nc.dram_tensor("msel", [B, T], F32, kind="ExternalInput")
    d_iotap = nc.dram_tensor("iotap", [V, 1], F32, kind="ExternalInput")
    d_ident = nc.dram_tensor("ident", [128, 128], F32, kind="ExternalInput")
    d_out = nc.dram_tensor("out", [B, HSL], F32, kind="ExternalOutput")

    # ---- persistent SBUF ----
    sb_ewb = nc.alloc_sbuf_tensor("sb_ewb", [V, G], XDT)
    sb_w0h = nc.alloc_sbuf_tensor("sb_w0h", [128, 8 * G], XDT)
    sb_w1 = nc.alloc_sbuf_tensor("sb_w1", [128, 16 * G], XDT)
    sb_b1 = nc.alloc_sbuf_tensor("sb_b1", [128, G], F32)
    sb_tok = nc.alloc_sbuf_tensor("sb_tok", [B, T], F32)
    sb_iota = nc.alloc_sbuf_tensor("sb_iota", [B, V], F32)
    sb_msel = nc.alloc_sbuf_tensor("sb_msel", [B, T], F32)
    sb_iotap = nc.alloc_sbuf_tensor("sb_iotap", [V, 1], F32)
    sb_ident = nc.alloc_sbuf_tensor("sb_ident", [128, 128], F32)
    c0 = nc.alloc_sbuf_tensor("c0", [B, HSL], F32)
    c1 = nc.alloc_sbuf_tensor("c1", [B, HSL], F32)
    h0bt = nc.alloc_sbuf_tensor("h0bt", [B, HSL], F32)
    h1bt = nc.alloc_sbuf_tensor("h1bt", [B, HSL], F32)
    zrow = nc.alloc_sbuf_tensor("zrow", [HSL, B], F32)
    outacc = nc.alloc_sbuf_tensor("outacc", [B, HSL], F32)

    with tile.TileContext(nc) as tc, ExitStack() as ctx:
        # ---- load weights/constants ----
        for sb, d in [(sb_ewb, d_ewb), (sb_w0h, d_w0h), (sb_w1, d_w1),
                      (sb_b1, d_b1), (sb_tok, d_tok), (sb_iota, d_iota),
                      (sb_msel, d_msel), (sb_iotap, d_iotap),
                      (sb_ident, d_ident)]:
            nc.sync.dma_start(sb[:], d[:])
        for st in (c0, c1, h0bt, h1bt, zrow, outacc):
            nc.vector.memset(st[:], 0.0)

        # ---- pools ----
        ps_z0 = ctx.enter_context(tc.tile_pool(name="psz0", bufs=2, space="PSUM"))
        ps_z1 = ctx.enter_context(tc.tile_pool(name="psz1", bufs=2, space="PSUM"))
        ps_tp = ctx.enter_context(tc.tile_pool(name="pstp", bufs=1, space="PSUM"))
        pool = ctx.enter_context(tc.tile_pool(name="work", bufs=3))
        rp0 = ctx.enter_context(tc.tile_pool(name="recv0", bufs=2))
        dram = ctx.enter_context(tc.tile_pool(name="dram", bufs=2, space="DRAM"))

        def do_ag(cin, cout):
            if ag_mode == "local":
                # timing-only: no exchange; block 0 fed from local cin
                nc.gpsimd.dma_start(cout[0:2 * HSL, :], cin[:])
            elif ag_mode == "fake":
                for j in range(NCORES):
                    nc.gpsimd.dma_start(
                        cout[j * 2 * HSL:(j + 1) * 2 * HSL, :], cin[:])
            else:
                nc.gpsimd.collective_compute(
                    "AllGather", ALU.bypass,
                    replica_groups=[list(range(NCORES))],
                    ins=[cin[:].opt()], outs=[cout[:].opt()],
                )

        # The two cells of one iteration (layer 0 of step t, layer 1 of
        # step t-1) are independent chains; their phases are emitted
        # interleaved so the in-order ACT/DVE queues never stall behind a
        # cross-engine wait belonging to the other cell (head-of-line
        # blocking). Phases: A = gate activations (ACT), B = c-update
        # (DVE), C = tanh(c) (ACT), D = h (DVE), then transposes (PE),
        # stage copies + capture (DVE, last — off the exchange path).

        # merged exchange: one AG/step carries [h0(t); h1(t-1)].
        # rh block layout: col (2j)*B..: core j's h0 slice; (2j+1)*B..: h1.
        def rblk0(j):
            return rh[:, (2 * j) * B:(2 * j) * B + B]

        def rblk1(j):
            return rh[:, (2 * j + 1) * B:(2 * j + 1) * B + B]

        rh = None

        for t in range(T + 1):
            # ---- one-hot for x_t, direct in [V, B] layout ----
            if t < T:
                ohbt = pool.tile([B, V], F32, tag="ohbt")
                nc.vector.tensor_scalar(
                    ohbt[:], sb_iota[:], sb_tok[:, t:t + 1], None,
                    ALU.is_equal)
                poh = ps_tp.tile([V, B], F32, tag="poh")
                nc.tensor.transpose(poh[:], ohbt[:], sb_ident[:])
                ohT = pool.tile([V, B], XDT, tag="ohT")
                nc.scalar.copy(ohT[:], poh[:])

                # ---- z0(t) = oh @ EWb + h0(t-1) @ W0h ----
                z0 = ps_z0.tile([B, G], F32, tag="z0")
                nc.tensor.matmul(z0[:], ohT[:], sb_ewb[:],
                                 start=True, stop=(t == 0))
                if t > 0:
                    for j in range(8):
                        nc.tensor.matmul(
                            z0[:], rblk0(j),
                            sb_w0h[:, j * G:(j + 1) * G],
                            start=False, stop=(j == 7))

            # ---- z1(t-1) = nh0(t-1) @ W1a + h1(t-2) @ W1b ----
            if t >= 1:
                z1 = ps_z1.tile([B, G], F32, tag="z1")
                for j in range(8):
                    nc.tensor.matmul(
                        z1[:], rblk0(j),
                        sb_w1[:, j * G:(j + 1) * G],
                        start=(j == 0), stop=(t == 1 and j == 7))
                if t >= 2:
                    for j in range(8):
                        nc.tensor.matmul(
                            z1[:], rblk1(j),
                            sb_w1[:, (8 + j) * G:(9 + j) * G],
                            start=False, stop=(j == 7),
                            skip_group_check=True)

            # ---- cells, phase-interleaved ----
            do0, do1 = t < T, t >= 1
            if do1:
                if has_b1:
                    zb = pool.tile([B, G], F32, tag="zb")
                    nc.vector.tensor_add(zb[:], z1[:], sb_b1[:])
                    z1ap = zb
                else:
                    z1ap = z1
            # phase A: gate activations (f-bias of layer 0 folded in EWb)
            if do0:
                sig0 = pool.tile([B, 3 * HSL], F32, tag="sig0")
                nc.scalar.activation(sig0[:], z0[:, 0:CJ], AF.Sigmoid)
                tanj0 = pool.tile([B, HSL], F32, tag="tanj0")
                nc.scalar.activation(tanj0[:], z0[:, CJ:CJ + HSL], AF.Tanh)
            if do1:
                sig1 = pool.tile([B, 3 * HSL], F32, tag="sig1")
                nc.scalar.activation(sig1[:, 0:CF], z1ap[:, 0:CF],
                                     AF.Sigmoid)
                nc.scalar.activation(sig1[:, CF:CJ], z1ap[:, CF:CJ],
                                     AF.Sigmoid, bias=1.0)
                tanj1 = pool.tile([B, HSL], F32, tag="tanj1")
                nc.scalar.activation(tanj1[:], z1ap[:, CJ:CJ + HSL],
                                     AF.Tanh)
            # phase B: c = c*sigf + sigi*tanj
            if do0:
                u0 = pool.tile([B, HSL], F32, tag="u0")
                nc.vector.tensor_mul(u0[:], sig0[:, 0:CO], tanj0[:])
                cm0 = pool.tile([B, HSL], F32, tag="cm0")
                nc.vector.tensor_mul(cm0[:], c0[:], sig0[:, CF:CJ])
                nc.vector.tensor_add(c0[:], cm0[:], u0[:])
            if do1:
                u1 = pool.tile([B, HSL], F32, tag="u1")
                nc.vector.tensor_mul(u1[:], sig1[:, 0:CO], tanj1[:])
                cm1 = pool.tile([B, HSL], F32, tag="cm1")
                nc.vector.tensor_mul(cm1[:], c1[:], sig1[:, CF:CJ])
                nc.vector.tensor_add(c1[:], cm1[:], u1[:])
            # phase C: tanh(c)
            if do0:
                tanc0 = pool.tile([B, HSL], F32, tag="tanc0")
                nc.scalar.activation(tanc0[:], c0[:], AF.Tanh)
            if do1:
                tanc1 = pool.tile([B, HSL], F32, tag="tanc1")
                nc.scalar.activation(tanc1[:], c1[:], AF.Tanh)
            # phase D: h = tanh(c)*sigo, then transposes + staging
            if do0:
                nc.vector.tensor_mul(h0bt[:], tanc0[:], sig0[:, CO:CF])
            if do1:
                nc.vector.tensor_mul(h1bt[:], tanc1[:], sig1[:, CO:CF])
            if do0:
                stg = pool.tile([HSL, 2 * B], XDT, tag="stg")
                tp0 = ps_tp.tile([HSL, B], F32, tag="tp0")
                nc.tensor.transpose(tp0[:], h0bt[:], sb_ident[:])
                if do1:
                    tp1 = ps_tp.tile([HSL, B], F32, tag="tp1")
                    nc.tensor.transpose(tp1[:], h1bt[:], sb_ident[:])
                nc.vector.tensor_copy(stg[:, 0:B], tp0[:])
                if do1:
                    nc.vector.tensor_copy(stg[:, B:2 * B], tp1[:])
                else:
                    nc.vector.tensor_copy(stg[:, B:2 * B], zrow[:])
            # capture h1(t-1) into the output at its firing step
            # (emitted last on DVE — not on the exchange critical path)
            if do1:
                nc.vector.scalar_tensor_tensor(
                    outacc[:], h1bt[:], sb_msel[:, t - 1:t], outacc[:],
                    ALU.mult, ALU.add)

            # ---- single AG + readback ----
            if t < T:
                cin = dram.tile([2 * HSL, B], XDT, tag="cin")
                nc.sync.dma_start(
                    cin[:].rearrange("(s p) b -> p s b", s=2),
                    stg[:].rearrange("p (s b) -> p s b", s=2))
                cout = dram.tile([NCORES * 2 * HSL, B], XDT, tag="cout",
                                 addr_space="Shared")
                do_ag(cin, cout)
                rh = rp0.tile([128, NCORES * 2 * HSL], XDT, tag="rh")
                nc.sync.dma_start(
                    rh[:].rearrange("p (j s b) -> p j s b", j=NCORES, s=2),
                    cout[:].rearrange("(j s p) b -> p j s b", j=NCORES, s=2))

        # ---- output ----
        nc.sync.dma_start(d_out[:], outacc[:])

    nc.compile()
    return nc


_CACHE = {}


def kernel(**inputs) -> np.ndarray:
    """Full-input entry point: returns [B, H] fp32 encoder output."""
    in_maps, has_b1 = _host_prep(inputs)
    key = ("nc", has_b1, EXCH)
    if key not in _CACHE:
        _CACHE[key] = build_kernel(has_b1=has_b1)
    nc = _CACHE[key]
    res = run_bass_kernel_spmd(nc, in_maps, core_ids=list(range(NCORES)))
    out = np.concatenate(
        [res.results[k]["out"] for k in range(NCORES)], axis=1)
    return out.astype(np.float32)



# revision 59
# speedup vs baseline: 1.1784x; 1.1286x over previous
"""nn_EncoderModel: 2-layer LSTM encoder (B=128, T=512, E=256, H=1024)
on 8 trn2 NeuronCores — v3.1.

Hidden-dim model parallelism (core k owns h-dims [128k,128(k+1)) of
both layers). Layer 1 is lagged TWO steps behind layer 0 and has its
own 32KB AllGather: cell1(t-2)'s inputs (gathered h0(t-2), h1(t-3))
come entirely from previous iterations' readbacks, so the whole
layer-1 block — z1 matmuls, cell1, its transpose and its AllGather —
runs at the TOP of each iteration, fully overlapped with the layer-0
exchange in flight. The critical recurrence cycle is only
readback(h0) -> z0 -> cell0 -> cin -> AG(h0), 32KB bf16 payload.

No per-step length masking: states evolve freely past each sequence's
end and the output is captured at its firing step
(outacc += msel_t * h1), mathematically identical to dynamic_rnn's
freeze-and-read-last.

Embedding lookup folds into the layer-0 matmul via the one-hot trick;
b0 and the layer-0 forget bias are folded into EWb on the host.

Queues: PE = z1 MMs, tp1, z0 MMs, tp0 per iteration (tp1 before z0 so
AG1 launches without waiting on this iteration's readback); ACT =
activations + ohT copy; DVE = cell elementwise + stage copies +
capture; gpsimd = AG1 then AG0; SP = cin DMAs + h0 readback; ACT HWDGE
carries the h1 readback to avoid head-of-line blocking.
AllGather outputs are addr_space="Shared" (the collective fast path).
"""

from contextlib import ExitStack

import numpy as np

import concourse.bass as bass
import concourse.mybir as mybir
import concourse.tile as tile
from concourse import bacc
from concourse.bass_utils import run_bass_kernel_spmd

F32 = mybir.dt.float32
F32R = mybir.dt.float32r
BF16 = mybir.dt.bfloat16
AF = mybir.ActivationFunctionType
ALU = mybir.AluOpType

B = 128      # batch (full, on every core)
E = 256      # embedding dim
H = 1024     # hidden
V = 128      # vocab
T = 512      # timesteps
HSL = 128    # hidden slice per core
G = 4 * HSL  # gate cols per core = 512
NCORES = 8

EXCH = "bf16"   # "f32r" | "bf16": dtype of weights + h-exchange payload

# gate order within each core's G columns: (i, o, f, j)
# reference order in W is (i, j, f, o) -> permutation of source blocks:
GATE_PERM = [0, 3, 2, 1]
CI, CO, CF, CJ = 0, 128, 256, 384  # column offsets of i/o/f/j blocks


def _np_dt(exch):
    if exch == "bf16":
        return mybir.dt.np(BF16)
    return np.float32


def _host_prep(inputs, exch=None):
    """Slice/transform full inputs into 8 per-core input maps."""
    exch = EXCH if exch is None else exch
    wdt = _np_dt(exch)
    ib = np.asarray(inputs["input_batch"])            # [B, T] int32
    lens = np.asarray(inputs["input_lengths"])        # [B]
    emb = np.asarray(inputs["char_embeddings"], dtype=np.float32)  # [V, E]
    W0 = np.asarray(inputs["W0"], dtype=np.float32)   # [E+H, 4H]
    b0 = np.asarray(inputs["b0"], dtype=np.float32)
    W1 = np.asarray(inputs["W1"], dtype=np.float32)   # [2H, 4H]
    b1 = np.asarray(inputs["b1"], dtype=np.float32)

    def gate_cols(W, k):
        return np.concatenate(
            [W[:, g * H + k * HSL: g * H + (k + 1) * HSL] for g in GATE_PERM],
            axis=1,
        )

    def gate_cols_b(b, k):
        return np.concatenate(
            [b[g * H + k * HSL: g * H + (k + 1) * HSL] for g in GATE_PERM]
        )

    tok_f32 = ib[:, :T].astype(np.float32)            # [B, T]
    iota_free = np.tile(np.arange(V, dtype=np.float32)[None, :], (B, 1))
    # capture mask: fires once, at each sequence's last valid step
    msel = (np.arange(T)[None, :] == (lens[:, None] - 1)).astype(np.float32)
    iota_p = np.arange(V, dtype=np.float32)[:, None]  # [V, 1]
    ident = np.eye(128, dtype=np.float32)

    in_maps = []
    for k in range(NCORES):
        W0c = gate_cols(W0, k)                        # [E+H, 512] (i,o,f,j)
        b0c = gate_cols_b(b0, k).copy()               # [512]
        b0c[CF:CF + HSL] += 1.0                       # forget bias layer 0
        ewb = emb @ W0c[:E] + b0c[None, :]            # [V, 512]
        w0h = W0c[E:]                                 # [1024, 512]
        w0h_t = np.concatenate(
            [w0h[j * 128: (j + 1) * 128] for j in range(8)], axis=1)
        W1c = gate_cols(W1, k)                        # [2048, 512]
        b1c = gate_cols_b(b1, k)
        # k-tile blocks: 0..7 = nh0 rows, 8..15 = h1 rows
        w1_t = np.concatenate(
            [W1c[j * 128: (j + 1) * 128] for j in range(16)], axis=1)
        b1_full = np.tile(b1c[None, :], (128, 1)).astype(np.float32)
        in_maps.append({
            "ewb": ewb.astype(wdt),
            "w0h": w0h_t.astype(wdt),
            "w1": w1_t.astype(wdt),
            "b1full": b1_full,
            "tok": tok_f32,
            "iotaf": iota_free,
            "msel": msel,
            "iotap": iota_p,
            "ident": ident,
        })
    has_b1 = bool(np.any(b1 != 0.0))
    return in_maps, has_b1


def build_kernel(has_b1=False, ag_mode="dual", exch=None):
    """Build + compile the SPMD Bass kernel for all 8 cores.

    ag_mode: "dual" (real AllGathers) / "fake" / "local" (no
    collectives; local copies — WRONG results, timing only).
    """
    exch = EXCH if exch is None else exch
    XDT = BF16 if exch == "bf16" else F32R
    nc = bacc.Bacc("TRN2", target_bir_lowering=False, debug=False,
                   num_devices=NCORES)

    # ---- I/O ----
    d_ewb = nc.dram_tensor("ewb", [V, G], XDT, kind="ExternalInput")
    d_w0h = nc.dram_tensor("w0h", [128, 8 * G], XDT, kind="ExternalInput")
    d_w1 = nc.dram_tensor("w1", [128, 16 * G], XDT, kind="ExternalInput")
    d_b1 = nc.dram_tensor("b1full", [128, G], F32, kind="ExternalInput")
    d_tok = nc.dram_tensor("tok", [B, T], F32, kind="ExternalInput")
    d_iota = nc.dram_tensor("iotaf", [B, V], F32, kind="ExternalInput")
    d_msel = nc.dram_tensor("msel", [B, T], F32, kind="ExternalInput")
    d_iotap = nc.dram_tensor("iotap", [V, 1], F32, kind="ExternalInput")
    d_ident = nc.dram_tensor("ident", [128, 128], F32, kind="ExternalInput")
    d_out = nc.dram_tensor("out", [B, HSL], F32, kind="ExternalOutput")

    # ---- persistent SBUF ----
    sb_ewb = nc.alloc_sbuf_tensor("sb_ewb", [V, G], XDT)
    sb_w0h = nc.alloc_sbuf_tensor("sb_w0h", [128, 8 * G], XDT)
    sb_w1 = nc.alloc_sbuf_tensor("sb_w1", [128, 16 * G], XDT)
    sb_b1 = nc.alloc_sbuf_tensor("sb_b1", [128, G], F32)
    sb_tok = nc.alloc_sbuf_tensor("sb_tok", [B, T], F32)
    sb_iota = nc.alloc_sbuf_tensor("sb_iota", [B, V], F32)
    sb_msel = nc.alloc_sbuf_tensor("sb_msel", [B, T], F32)
    sb_iotap = nc.alloc_sbuf_tensor("sb_iotap", [V, 1], F32)
    sb_ident = nc.alloc_sbuf_tensor("sb_ident", [128, 128], F32)
    c0 = nc.alloc_sbuf_tensor("c0", [B, HSL], F32)
    c1 = nc.alloc_sbuf_tensor("c1", [B, HSL], F32)
    h0bt = nc.alloc_sbuf_tensor("h0bt", [B, HSL], F32)
    h1bt = nc.alloc_sbuf_tensor("h1bt", [B, HSL], F32)
    outacc = nc.alloc_sbuf_tensor("outacc", [B, HSL], F32)

    with tile.TileContext(nc) as tc, ExitStack() as ctx:
        # ---- load weights/constants ----
        for sb, d in [(sb_ewb, d_ewb), (sb_w0h, d_w0h), (sb_w1, d_w1),
                      (sb_b1, d_b1), (sb_tok, d_tok), (sb_iota, d_iota),
                      (sb_msel, d_msel), (sb_iotap, d_iotap),
                      (sb_ident, d_ident)]:
            nc.sync.dma_start(sb[:], d[:])
        for st in (c0, c1, h0bt, h1bt, outacc):
            nc.vector.memset(st[:], 0.0)

        # ---- pools ----
        ps_z0 = ctx.enter_context(tc.tile_pool(name="psz0", bufs=2, space="PSUM"))
        ps_z1 = ctx.enter_context(tc.tile_pool(name="psz1", bufs=2, space="PSUM"))
        ps_tp = ctx.enter_context(tc.tile_pool(name="pstp", bufs=1, space="PSUM"))
        pool = ctx.enter_context(tc.tile_pool(name="work", bufs=3))
        rp0 = ctx.enter_context(tc.tile_pool(name="recv0", bufs=3))
        rp1 = ctx.enter_context(tc.tile_pool(name="recv1", bufs=2))
        dram = ctx.enter_context(tc.tile_pool(name="dram", bufs=2, space="DRAM"))

        def do_ag(cin, cout):
            if ag_mode == "local":
                nc.gpsimd.dma_start(cout[0:HSL, :], cin[:])
            elif ag_mode == "fake":
                for j in range(NCORES):
                    nc.gpsimd.dma_start(
                        cout[j * HSL:(j + 1) * HSL, :], cin[:])
            else:
                nc.gpsimd.collective_compute(
                    "AllGather", ALU.bypass,
                    replica_groups=[list(range(NCORES))],
                    ins=[cin[:].opt()], outs=[cout[:].opt()],
                )

        def cell(z, cst, hst, layer):
            """Pure LSTM cell on PSUM gates z [B, G] in (i,o,f,j) order;
            updates cst/hst in place (no length masking)."""
            sig = pool.tile([B, 3 * HSL], F32, tag=f"sig{layer}")
            if layer == 0:
                # f-bias folded into EWb: one fused sigmoid over i|o|f
                nc.scalar.activation(sig[:], z[:, 0:CJ], AF.Sigmoid)
            else:
                nc.scalar.activation(sig[:, 0:CF], z[:, 0:CF], AF.Sigmoid)
                nc.scalar.activation(sig[:, CF:CJ], z[:, CF:CJ], AF.Sigmoid,
                                     bias=1.0)
            tanj = pool.tile([B, HSL], F32, tag=f"tanj{layer}")
            nc.scalar.activation(tanj[:], z[:, CJ:CJ + HSL], AF.Tanh)
            # c = c*sigf + sigi*tanj
            u = pool.tile([B, HSL], F32, tag=f"u{layer}")
            nc.vector.tensor_mul(u[:], sig[:, 0:CO], tanj[:])
            cm = pool.tile([B, HSL], F32, tag=f"cm{layer}")
            nc.vector.tensor_mul(cm[:], cst, sig[:, CF:CJ])
            nc.vector.tensor_add(cst, cm[:], u[:])
            # h = tanh(c) * sigo
            tanc = pool.tile([B, HSL], F32, tag=f"tanc{layer}")
            nc.scalar.activation(tanc[:], cst, AF.Tanh)
            nc.vector.tensor_mul(hst, tanc[:], sig[:, CO:CF])

        # rh0_cur = gathered h0(t-1); rh0_prev = gathered h0(t-2);
        # rh1_cur = gathered h1(t-3) (from AG1 of iteration t-1).
        rh0_cur = rh0_prev = rh1_cur = None

        for t in range(T + 2):
            # ---- one-hot + EWb matmul first: no readback dependency, so
            # PE starts the iteration immediately ----
            if t < T:
                ohbt = pool.tile([B, V], F32, tag="ohbt")
                nc.vector.tensor_scalar(
                    ohbt[:], sb_iota[:], sb_tok[:, t:t + 1], None,
                    ALU.is_equal)
                poh = ps_tp.tile([V, B], F32, tag="poh")
                nc.tensor.transpose(poh[:], ohbt[:], sb_ident[:])
                ohT = pool.tile([V, B], XDT, tag="ohT")
                nc.scalar.copy(ohT[:], poh[:])
                z0 = ps_z0.tile([B, G], F32, tag="z0")
                nc.tensor.matmul(z0[:], ohT[:], sb_ewb[:],
                                 start=True, stop=(t == 0))

            # ---- layer 1 for step t-2: inputs are all from history, so
            # this block (and AG1) runs before/under AG0's round trip ----
            if t >= 2:
                z1 = ps_z1.tile([B, G], F32, tag="z1")
                for j in range(8):
                    nc.tensor.matmul(
                        z1[:], rh0_prev[:, j * B:(j + 1) * B],
                        sb_w1[:, j * G:(j + 1) * G],
                        start=(j == 0), stop=(t == 2 and j == 7))
                if t >= 3:
                    for j in range(8):
                        nc.tensor.matmul(
                            z1[:], rh1_cur[:, j * B:(j + 1) * B],
                            sb_w1[:, (8 + j) * G:(9 + j) * G],
                            start=False, stop=(j == 7),
                            skip_group_check=True)
                if has_b1:
                    zb = pool.tile([B, G], F32, tag="zb")
                    nc.vector.tensor_add(zb[:], z1[:], sb_b1[:])
                    z1ap = zb
                else:
                    z1ap = z1
                cell(z1ap, c1[:], h1bt[:], 1)
                # capture h1(t-2) into the output at its firing step
                nc.vector.scalar_tensor_tensor(
                    outacc[:], h1bt[:], sb_msel[:, t - 2:t - 1], outacc[:],
                    ALU.mult, ALU.add)
                if t <= T:
                    tp1 = ps_tp.tile([HSL, B], F32, tag="tp1")
                    nc.tensor.transpose(tp1[:], h1bt[:], sb_ident[:])
                    stg1 = pool.tile([HSL, B], XDT, tag="stg1")
                    nc.vector.tensor_copy(stg1[:], tp1[:])
                    cin1 = dram.tile([HSL, B], XDT, tag="cin1")
                    nc.sync.dma_start(cin1[:], stg1[:])
                    cout1 = dram.tile([NCORES * HSL, B], XDT, tag="cout1",
                                      addr_space="Shared")
                    do_ag(cin1, cout1)

            # ---- layer 0 for step t: z0 h-matmuls, cell0, AG0 ----
            if t < T:
                if t > 0:
                    for j in range(8):
                        nc.tensor.matmul(
                            z0[:], rh0_cur[:, j * B:(j + 1) * B],
                            sb_w0h[:, j * G:(j + 1) * G],
                            start=False, stop=(j == 7),
                            skip_group_check=True)
                cell(z0, c0[:], h0bt[:], 0)
                tp0 = ps_tp.tile([HSL, B], F32, tag="tp0")
                nc.tensor.transpose(tp0[:], h0bt[:], sb_ident[:])
                stg0 = pool.tile([HSL, B], XDT, tag="stg0")
                nc.vector.tensor_copy(stg0[:], tp0[:])
                cin0 = dram.tile([HSL, B], XDT, tag="cin0")
                nc.sync.dma_start(cin0[:], stg0[:])
                cout0 = dram.tile([NCORES * HSL, B], XDT, tag="cout0",
                                  addr_space="Shared")
                do_ag(cin0, cout0)

            # ---- readbacks: h0 on SP (critical), h1 on ACT HWDGE ----
            if t < T:
                rh0n = rp0.tile([128, NCORES * B], XDT, tag="rh0")
                nc.sync.dma_start(
                    rh0n[:].rearrange("p (j b) -> p j b", j=NCORES),
                    cout0[:].rearrange("(j p) b -> p j b", j=NCORES))
            else:
                rh0n = None
            if 2 <= t <= T:
                rh1n = rp1.tile([128, NCORES * B], XDT, tag="rh1")
                nc.scalar.dma_start(
                    rh1n[:].rearrange("p (j b) -> p j b", j=NCORES),
                    cout1[:].rearrange("(j p) b -> p j b", j=NCORES))
                rh1_cur = rh1n
            rh0_prev = rh0_cur
            rh0_cur = rh0n

        # ---- output ----
        nc.sync.dma_start(d_out[:], outacc[:])

    nc.compile()
    return nc


_CACHE = {}


def kernel(**inputs) -> np.ndarray:
    """Full-input entry point: returns [B, H] fp32 encoder output."""
    in_maps, has_b1 = _host_prep(inputs)
    key = ("nc", has_b1, EXCH)
    if key not in _CACHE:
        _CACHE[key] = build_kernel(has_b1=has_b1)
    nc = _CACHE[key]
    res = run_bass_kernel_spmd(nc, in_maps, core_ids=list(range(NCORES)))
    out = np.concatenate(
        [res.results[k]["out"] for k in range(NCORES)], axis=1)
    return out.astype(np.float32)
